# revision 1
# baseline (speedup 1.0000x reference)
"""GatedDeltaNet (B=2, T=1024, D=512, H=1) fully on-device on 8 trn2 cores.

Sharding: core (b, s) = batch b x Dv-slice s (128 v-columns of the state).
The sequential delta-rule scan parallelizes over Dv with no cross-core
traffic; only the final RMSNorm sum and output projection cross slices.

Single launch per 8 cores:
  * bf16 projections (q,k full + v,gate slices) via PE, causal dwconv as
    diagonal-matmuls on PE, silu via the HW act table.
  * l2norm/decay/beta scalars computed in [128, 8] column space; sumsq
    taken from the diagonals of the K K^T / Q Q^T chunk Gram matrices.
  * chunked (C=128) gated delta rule: within-chunk (I+M)^{-1} via the
    exact log-depth Neumann factorization truncated at (I-M)(I+M^2);
    state updates through PE-transposed K chunks; all chunk operands bf16
    with f32 PSUM accumulation.
  * per-chunk partial output projections with the RMSNorm row scale
    commuted past the matmul; a single 4-core ReduceScatter (partials
    + packed sumsq column) hands each core its row-quarter, normalized
    on-device.
HW exec time is the genuine NTFF-profiled device execution time
(max traced core), captured via the axon NRT profile hook.
"""

import sys
import types
import time
from contextlib import ExitStack

import numpy as np
import ml_dtypes

BF16 = ml_dtypes.bfloat16

B, T, D, KC = 2, 1024, 512, 4
P = 128
C = 128          # chunk length
NCH = T // C     # 8 chunks
NDT = D // P     # 4 dk tiles
NLVL = 6         # Neumann levels: M^2..M^64

_LAST_HW_NS = [None]
USE_SILU_TABLE = True   # real HW has a silu table; CoreSim does not


# ─────────────────────────── axon NTFF hook shim ──────────────────────────
def install_ntff_shim():
    if "antenv.axon_hooks" in sys.modules:
        return
    try:
        import antenv  # noqa: F401
        from trn_agent_boot.trn_boot import _ntff_profile_via_ctypes
        hook = _ntff_profile_via_ctypes('/opt/axon/libaxon_pjrt.so')
    except Exception:
        hook = None
    mod = types.ModuleType("antenv.axon_hooks")
    mod.get_axon_ntff_profile_hook = lambda: hook
    mod.set_axon_ntff_profile_hook = lambda h: None
    sys.modules["antenv.axon_hooks"] = mod


# ─────────────────────────────── L1 kernel ────────────────────────────────
def l1_kernel(tc, ins, outs):
    import concourse.bass as bass
    import concourse.mybir as mybir
    from concourse.masks import make_identity

    nc = tc.nc
    fp32 = mybir.dt.float32
    bf16 = mybir.dt.bfloat16
    AF = mybir.ActivationFunctionType
    OP = mybir.AluOpType

    xT, wqk, wv, wba, convw, sc = (ins[k] for k in
                                   ("xT", "wqk", "wv", "wba", "convw", "sc"))
    wgs, wos, wnr = (ins[k] for k in ("wgs", "wos", "wnr"))
    out_rows = outs["out"]

    ctx = ExitStack()
    with ctx:
        sing = ctx.enter_context(tc.tile_pool(name="sing", bufs=1))
        sb2 = ctx.enter_context(tc.tile_pool(name="sb2", bufs=2))
        ps = ctx.enter_context(tc.tile_pool(name="ps", bufs=1, space="PSUM"))
        ps2 = ctx.enter_context(tc.tile_pool(name="ps2", bufs=2, space="PSUM"))
        dramp = ctx.enter_context(tc.tile_pool(name="dramp", bufs=1, space="DRAM"))

        # ── load inputs (two pieces per big tensor) ──
        xT_sb = sing.tile([P, NDT * 1024], bf16)
        xT_v = xT.rearrange("p (k t) -> p k t", k=NDT)
        xTs_v = xT_sb.rearrange("p (k t) -> p k t", k=NDT)
        for kt, eng_ in ((0, nc.sync), (1, nc.scalar), (2, nc.gpsimd),
                         (3, nc.sync)):
            eng_.dma_start(xTs_v[:, kt], xT_v[:, kt])
        wqk_sb = sing.tile([P, NDT * 1024], bf16)
        wqk_v = wqk.rearrange("p (k t) -> p k t", k=NDT)
        wqks_v = wqk_sb.rearrange("p (k t) -> p k t", k=NDT)
        for th, eng_ in ((0, nc.gpsimd), (1, nc.scalar)):
            eng_.dma_start(wqks_v[:, :, th * 512:(th + 1) * 512],
                           wqk_v[:, :, th * 512:(th + 1) * 512])
        wv_sb = sing.tile([P, NDT * P], bf16)
        nc.sync.dma_start(wv_sb, wv)
        wg_sb = sing.tile([P, NDT * P], bf16)
        nc.sync.dma_start(wg_sb, wgs)
        wo_sb = sing.tile([P, 512], bf16)
        nc.sync.dma_start(wo_sb, wos)
        wnr_sb = sing.tile([1, P], fp32)
        nc.sync.dma_start(wnr_sb, wnr)
        wnormB = sing.tile([P, P], fp32)
        nc.gpsimd.partition_broadcast(wnormB, wnr_sb)
        eps5c = sing.tile([P, 1], fp32)
        nc.vector.memset(eps5c, 1e-5)
        wba_sb = sing.tile([P, NDT * 2], bf16)
        nc.sync.dma_start(wba_sb, wba)
        convw_sb = sing.tile([P, 36], fp32)
        nc.sync.dma_start(convw_sb, convw)
        sc_sb = sing.tile([1, 8], fp32)
        nc.sync.dma_start(sc_sb, sc)

        from concourse.tile import add_dep_helper as _adh
        act_order = []
        logexp_late = []
        ident = sing.tile([P, P], fp32)
        make_identity(nc, ident)
        one1 = sing.tile([1, 1], fp32)
        nc.vector.memset(one1, 1.0)
        ones11 = sing.tile([1, 1], fp32)
        nc.vector.memset(ones11, 1.0)
        ones128 = sing.tile([P, P], fp32)
        nc.vector.memset(ones128, 1.0)
        epsc = sing.tile([P, 1], fp32)
        nc.vector.memset(epsc, 1e-6)
        lnrq = sing.tile([P, 1], fp32)
        nc.vector.memset(lnrq, float(-0.5 * np.log(D)))
        # one-hot row 127 selector
        oh127 = sing.tile([P, 1], fp32)
        nc.vector.memset(oh127, 1.0)
        nc.gpsimd.affine_select(oh127, oh127, [[0, 1]], OP.is_equal, 0.0,
                                base=-127, channel_multiplier=1)

        # conv-as-matmul: per (group, tap) diagonal weight tiles, built on ACT
        ident_bf = sing.tile([P, P], bf16)
        nc.vector.tensor_copy(ident_bf, ident)
        diag4 = [sing.tile([P, 4, P], bf16, tag=f"diag4_{g}", name=f"diag4_{g}")
                 for g in range(9)]
        diag = {}
        for g in range(9):
            for j in range(4):

                nc.vector.tensor_scalar_mul(diag4[g][:, j, :], ident_bf,
                                            convw_sb[:, 4 * g + j:4 * g + j + 1])

        # ── projections + conv pads ──
        pads = {}
        for name, n_dt in (("q", NDT), ("k", NDT), ("v", 1)):
            for dt_i in range(n_dt):
                pad = sing.tile([P, 3 + 1024], bf16, tag=f"pad_{name}{dt_i}",
                                name=f"pad_{name}{dt_i}")
                nc.vector.memset(pad[:, 0:3], 0.0)
                pads[(name, dt_i)] = pad

        def proj_cols(name, dt_i, kt):
            if name == "q":
                return wqk_sb[:, kt * 1024 + dt_i * P: kt * 1024 + (dt_i + 1) * P]
            if name == "k":
                return wqk_sb[:, kt * 1024 + 512 + dt_i * P:
                              kt * 1024 + 512 + (dt_i + 1) * P]
            return wv_sb[:, kt * P:(kt + 1) * P]

        wqk3 = wqk_sb.rearrange("p (k t) -> p k t", k=NDT)
        wv3 = wv_sb.rearrange("p (k t) -> p k t", k=NDT)
        xT3 = xT_sb.rearrange("p (k t) -> p k t", k=NDT)
        ecnt = 0
        for th in range(2):  # t-half, 512 cols
            for name, n_dt in (("q", NDT), ("k", NDT), ("v", 1)):
                for dt_i in range(n_dt):
                    pp = ps2.tile([P, 512], fp32, tag="big", name="pp", bufs=3)
                    for kt in range(NDT):
                        if name == "q":
                            lhs_ = wqk3[:, kt, dt_i * P:(dt_i + 1) * P]
                        elif name == "k":
                            lhs_ = wqk3[:, kt, 512 + dt_i * P: 512 + (dt_i + 1) * P]
                        else:
                            lhs_ = wv3[:, kt]
                        nc.tensor.matmul(
                            pp, lhs_, xT3[:, kt, th * 512:(th + 1) * 512],
                            start=(kt == 0), stop=(kt == NDT - 1))
                    eng2 = nc.scalar if (ecnt % 4 == 0) else nc.vector
                    ecnt += 1
                    if eng2 is nc.scalar:
                        nc.scalar.copy(
                            pads[(name, dt_i)][:, 3 + th * 512: 3 + (th + 1) * 512], pp)
                    else:
                        nc.vector.tensor_copy(
                            pads[(name, dt_i)][:, 3 + th * 512: 3 + (th + 1) * 512], pp)

        # beta / a rows
        softplus_insts = []
        blin_row = sing.tile([1, 1024], fp32)
        grow = sing.tile([1, 1024], fp32)
        sp = sing.tile([1, 1024], fp32, tag="sprow", name="sprow")
        sp2 = sing.tile([1, 1024], fp32, tag="sp2row", name="sp2row")
        wba3 = wba_sb.rearrange("p (k t) -> p k t", k=NDT)
        for th in range(2):
            bp = ps2.tile([1, 512], fp32, tag="big", name="bp", bufs=3)
            ap_ = ps2.tile([1, 512], fp32, tag="big", name="ap_", bufs=3)
            for kt in range(NDT):
                xs_ = xT3[:, kt, th * 512:(th + 1) * 512]
                nc.tensor.matmul(bp, wba3[:, kt, 0:1], xs_,
                                 start=(kt == 0), stop=(kt == NDT - 1))
                nc.tensor.matmul(ap_, wba3[:, kt, 1:2], xs_,
                                 start=(kt == 0), stop=(kt == NDT - 1))
            sl = slice(th * 512, (th + 1) * 512)
            nc.scalar.copy(blin_row[:, sl], bp)
            # softplus(a + dt_bias) = ln(1 + exp(a + dt_bias)); exp now, ln below
            nc.scalar.activation(sp[:, sl], ap_, AF.Exp,
                                 bias=sc_sb[:, 0:1], scale=1.0)
        for th in range(2):
            sl = slice(th * 512, (th + 1) * 512)
            ln_i = nc.scalar.activation(sp2[:, sl], sp[:, sl], AF.Ln,
                                        bias=one1[:, 0:1], scale=1.0)
            softplus_insts.append(ln_i)
            nc.vector.tensor_scalar_mul(grow[:, sl], sp2[:, sl], sc_sb[:, 1:2])

        # conv (4 taps) + silu (x*sigmoid).
        # kq_all[p, i, c, 0:128]=K-chunk, [...,128:256]=Q-chunk
        kq_all = sing.tile([P, NDT, NCH, 2 * P], bf16)
        kqcat = [kq_all[:, i] for i in range(NDT)]
        vTf = sing.tile([P, 1024], bf16)

        def kslice(i, c):
            return kq_all[:, i, c, 0:P]

        def qslice(i, c):
            return kq_all[:, i, c, P:2 * P]

        grp_idx = {("q", 0): 0, ("q", 1): 1, ("q", 2): 2, ("q", 3): 3,
                   ("k", 0): 4, ("k", 1): 5, ("k", 2): 6, ("k", 3): 7,
                   ("v", 0): 8}
        for (name, dt_i), pad in pads.items():
            g = grp_idx[(name, dt_i)]
            eng = nc.vector if (g % 2 == 0) else nc.gpsimd
            for th in range(2):
                cps = ps2.tile([P, 512], fp32, tag="big", name="cps", bufs=3)
                for j in range(4):
                    nc.tensor.matmul(cps, diag4[g][:, j],
                                     pad[:, th * 512 + j: th * 512 + j + 512],
                                     start=(j == 0), stop=(j == 3))
                cv = cps.rearrange("p (c t) -> p c t", c=4)
                crange = slice(th * 4, (th + 1) * 4)
                if USE_SILU_TABLE:
                    if name == "v":
                        act_order.append(nc.scalar.activation(
                            vTf[:, th * 512:(th + 1) * 512], cps, AF.Silu))
                    else:
                        off = 0 if name == "k" else P
                        act_order.append(nc.scalar.activation(
                            kqcat[dt_i][:, crange, off:off + P], cv, AF.Silu))
                else:
                    sg = sb2.tile([P, 512], bf16, tag="sgc", name="sgc")
                    act_order.append(nc.scalar.activation(sg, cps, AF.Sigmoid))
                    sgv = sg.rearrange("p (c t) -> p c t", c=4)
                    if name == "v":
                        nc.vector.tensor_mul(vTf[:, th * 512:(th + 1) * 512],
                                             cps, sg)
                    else:
                        off = 0 if name == "k" else P
                        nc.vector.tensor_mul(kqcat[dt_i][:, crange, off:off + P],
                                             cv, sgv)

        # ── L row (cumsum of g per chunk) ──
        ones_row = sing.tile([1, C], fp32)
        nc.vector.memset(ones_row, 1.0)
        Lrow = sing.tile([1, 1024], fp32)
        for c in range(NCH):
            nc.vector.tensor_tensor_scan(
                Lrow[:, c * C:(c + 1) * C], ones_row, grow[:, c * C:(c + 1) * C],
                0.0, OP.mult, OP.add)

        # rows -> columns via K=1 transpose matmuls
        bl_col = sing.tile([P, NCH], fp32)
        L_col = sing.tile([P, NCH], fp32)
        for vec_row, col in ((blin_row, bl_col), (Lrow, L_col)):
            cp = ps2.tile([P, NCH], fp32, tag="big", name="colp", bufs=3)
            for c in range(NCH):
                nc.tensor.matmul(cp[:, c:c + 1], vec_row[:, c * C:(c + 1) * C],
                                 ones11, start=True, stop=True)
            nc.vector.tensor_copy(col, cp)

        # ── chunk pass A: KK/KQ (stored) + QQ; diag -> sumsq cols ──
        kkq_sb = [sing.tile([P, 2 * P], bf16, tag=f"kkq{c}", name=f"kkq{c}")
                  for c in range(NCH)]
        ssqk_col = sing.tile([P, NCH], fp32)
        ssqq_col = sing.tile([P, NCH], fp32)
        for c in range(NCH):
            kkq = ps2.tile([P, 3 * P], fp32, tag="big", name="kkq", bufs=3)
            for i in range(NDT):
                nc.tensor.matmul(kkq[:, 0:2 * P], kq_all[:, i, c, 0:P],
                                 kq_all[:, i, c, :],
                                 start=(i == 0), stop=(i == NDT - 1))
            for i in range(NDT):
                nc.tensor.matmul(kkq[:, 2 * P:3 * P], kq_all[:, i, c, P:2 * P],
                                 kq_all[:, i, c, P:2 * P],
                                 start=(i == 0), stop=(i == NDT - 1))
            nc.vector.tensor_copy(kkq_sb[c], kkq[:, 0:2 * P])
            junk = sb2.tile([P, P], bf16, tag="junk", name="junk")
            nc.vector.scalar_tensor_tensor(junk, kkq_sb[c][:, 0:P], 1.0, ident,
                                           OP.mult, OP.mult,
                                           accum_out=ssqk_col[:, c:c + 1])
            junk2 = sb2.tile([P, P], bf16, tag="junk", name="junk2")
            nc.vector.scalar_tensor_tensor(junk2, kkq[:, 2 * P:3 * P], 1.0, ident,
                                           OP.mult, OP.mult,
                                           accum_out=ssqq_col[:, c:c + 1])

        # ── column-space scalar pipeline [128, 8] ──
        bcol = sing.tile([P, NCH], fp32)
        bexp = sing.tile([P, NCH], fp32)
        logexp_late.append(nc.scalar.activation(bexp, bl_col, AF.Exp,
                                                scale=-1.0))
        bexp1 = sing.tile([P, NCH], fp32)
        nc.vector.tensor_scalar_add(bexp1, bexp, 1.0)
        nc.vector.reciprocal(bcol, bexp1)
        lam_col = sing.tile([P, NCH], fp32)
        eneg_col = sing.tile([P, NCH], fp32)
        # rk = exp(-0.5 ln(ssq+eps)); rq = exp(-0.5 ln(ssq+eps) - 0.5 ln D)
        lnk = sing.tile([P, NCH], fp32)
        logexp_late.append(nc.scalar.activation(lnk, ssqk_col, AF.Ln,
                                                bias=epsc[:, 0:1]))
        lnq = sing.tile([P, NCH], fp32)
        logexp_late.append(nc.scalar.activation(lnq, ssqq_col, AF.Ln,
                                                bias=epsc[:, 0:1]))
        logexp_late.append(nc.scalar.activation(lam_col, L_col, AF.Exp))
        logexp_late.append(nc.scalar.activation(eneg_col, L_col, AF.Exp,
                                                scale=-1.0))
        rk_col = sing.tile([P, NCH], fp32)
        logexp_late.append(nc.scalar.activation(rk_col, lnk, AF.Exp, scale=-0.5))
        rq_col = sing.tile([P, NCH], fp32)
        logexp_late.append(nc.scalar.activation(rq_col, lnq, AF.Exp, scale=-0.5,
                                                bias=lnrq[:, 0:1]))
        # ACT table grouping: softplus(ln/exp) -> sigmoids -> late ln/exp
        for si in act_order:
            for li in softplus_insts:
                _adh(si.ins, li.ins, sync=False,
                     reason="act-table grouping: sigmoid after softplus")
        for ei in logexp_late:
            for si in act_order:
                _adh(ei.ins, si.ins, sync=False,
                     reason="act-table grouping: late logexp after sigmoids")
        colf_col = sing.tile([P, NCH], fp32)
        nc.vector.tensor_mul(colf_col, eneg_col, rk_col)
        qf_col = sing.tile([P, NCH], fp32)
        nc.vector.tensor_mul(qf_col, lam_col, rq_col)
        rowfM_col = sing.tile([P, NCH], fp32)
        nc.vector.tensor_mul(rowfM_col, bcol, lam_col)
        nc.vector.tensor_mul(rowfM_col, rowfM_col, rk_col)
        rowfMn_col = sing.tile([P, NCH], fp32)
        nc.vector.tensor_scalar_mul(rowfMn_col, rowfM_col, -1.0)
        # lam per chunk: lam8row = oh127^T @ lam_col, bcast
        l8p = ps2.tile([1, NCH], fp32, tag="big", name="l8p", bufs=3)
        nc.tensor.matmul(l8p, oh127, lam_col, start=True, stop=True)
        lam8 = sing.tile([1, NCH], fp32)
        nc.vector.tensor_copy(lam8, l8p)
        lamB = sing.tile([P, NCH], fp32)
        nc.gpsimd.partition_broadcast(lamB, lam8)
        kbar_col = sing.tile([P, NCH], fp32)
        nc.vector.tensor_mul(kbar_col, colf_col, lamB)

        # ── V transpose + beta scale ──
        Vb = [sing.tile([P, P], bf16, tag=f"Vb{c}", name=f"Vb{c}")
              for c in range(NCH)]
        for c in range(NCH):
            tp = ps2.tile([P, 2 * P], bf16, tag="pair", name="vtp")[:, 0:P]
            nc.tensor.transpose(tp, vTf[:, c * C:(c + 1) * C], ident_bf)
            nc.vector.tensor_scalar_mul(Vb[c], tp, bcol[:, c:c + 1])

        # ── K natural-layout [t, d] chunks (for the state update) ──
        Knat = [sing.tile([P, D], bf16, tag=f"Knat{c}", name=f"Knat{c}")
                for c in range(NCH)]
        for c in range(NCH):
            for half in range(2):
                tpk = ps2.tile([P, 2 * P], bf16, tag="pair", name="tpk")
                nc.tensor.transpose(tpk[:, 0:P], kslice(2 * half, c), ident_bf)
                nc.tensor.transpose(tpk[:, P:2 * P], kslice(2 * half + 1, c), ident_bf)
                nc.vector.tensor_copy(Knat[c][:, half * 2 * P:(half + 1) * 2 * P], tpk)

        # ── chunk pass B: M, MT, M2 pair, TT, AmatT ──
        AmatT = [sing.tile([P, P], bf16, tag=f"Am{c}", name=f"Am{c}")
                 for c in range(NCH)]
        TTs = [sing.tile([P, P], bf16, tag=f"TT{c}", name=f"TT{c}")
               for c in range(NCH)]
        for c in range(NCH):
            # colfB chunk tile: replicate colf col -> transpose
            crep = sb2.tile([P, P], bf16, tag="crep", name="crep")
            nc.scalar.mul(crep, ones128, colf_col[:, c:c + 1])
            cm = ps2.tile([P, 2 * P], bf16, tag="pair", name="cm")
            cbp = cm[:, 0:P]
            mtp = cm[:, P:2 * P]
            nc.tensor.transpose(cbp, crep, ident_bf)
            # M [t,i] strict lower
            M = sb2.tile([P, P], bf16, tag="M", name="M")
            nc.vector.scalar_tensor_tensor(M, kkq_sb[c][:, 0:P],
                                           rowfM_col[:, c:c + 1],
                                           cbp, OP.mult, OP.mult)
            nc.gpsimd.affine_select(M, M, [[-1, P]], OP.is_ge, 0.0,
                                    base=-1, channel_multiplier=1)
            # MT via PE transpose (already masked)
            nc.tensor.transpose(mtp, M, ident_bf)
            MT = sb2.tile([P, P], bf16, tag="MT", name="MT")
            nc.vector.tensor_copy(MT, mtp)
            # AmatT [i,t] upper incl diag
            nc.vector.tensor_scalar_mul(AmatT[c], kkq_sb[c][:, P:2 * P],
                                        colf_col[:, c:c + 1])
            nc.gpsimd.affine_select(AmatT[c], AmatT[c], [[1, P]], OP.is_ge,
                                    0.0, base=0, channel_multiplier=-1)
            # M2 pair
            pr = ps2.tile([P, 2 * P], fp32, tag="pair", name="pair")
            nc.tensor.matmul(pr[:, 0:P], MT, M, start=True, stop=True)
            nc.tensor.matmul(pr[:, P:2 * P], M, MT, start=True, stop=True)
            pw0 = sb2.tile([P, 2 * P], bf16, tag="pw0", name="pw0")
            nc.vector.tensor_copy(pw0, pr)
            # TT = (I + M2T)(I - MT)
            tt = sb2.tile([P, P], bf16, tag="ttp", name="ttp")
            nc.vector.tensor_sub(tt, ident, MT)
            tp_ = ps2.tile([P, P], fp32, tag="pair", name="ttps")
            nc.tensor.matmul(tp_, pw0[:, 0:P], tt, start=True, stop=True)
            nc.vector.tensor_add(TTs[c], tt, tp_)

        # ── serial scan ──
        S_t = sing.tile([P, NDT, P], bf16, tag="St_1", name="St_init")
        nc.vector.memset(S_t, 0.0)
        o_sb = [sing.tile([P, P], bf16, tag=f"osb{c}", name=f"osb{c}")
                for c in range(NCH)]
        ssq_sb = sing.tile([P, NCH], fp32)
        part = dramp.tile([NCH * P, 520], bf16, name="part")

        for c in range(NCH):
            cs = slice(c * C, (c + 1) * C)
            hp = ps.tile([P, 3 * P], fp32, tag="h", name="hp", bufs=3)
            ksp = hp[:, 0:P]
            wp = hp[:, P:2 * P]
            op_ = hp[:, 2 * P:3 * P]
            for i in range(NDT):
                nc.tensor.matmul(ksp, kq_all[:, i, c, 0:P], S_t[:, i],
                                 start=(i == 0), stop=(i == NDT - 1))
            rhsw = sb2.tile([P, P], bf16, tag="rhsw", name="rhsw")
            nc.vector.scalar_tensor_tensor(rhsw, ksp, rowfMn_col[:, c:c + 1],
                                           Vb[c], OP.mult, OP.add)
            nc.tensor.matmul(wp, TTs[c], rhsw, start=True, stop=True)
            W = sb2.tile([P, P], bf16, tag="W", name="W")
            nc.vector.tensor_copy(W, wp)
            W2 = sb2.tile([P, P], bf16, tag="W2", name="W2")
            nc.vector.tensor_scalar_mul(W2, wp, kbar_col[:, c:c + 1])
            # o = qf * (sum_d QT.T S + AmatT.T W)
            for i in range(NDT):
                nc.tensor.matmul(op_, kq_all[:, i, c, P:2 * P], S_t[:, i],
                                 start=(i == 0), stop=False)
            nc.tensor.matmul(op_, AmatT[c], W, start=False, stop=True)
            nc.scalar.mul(o_sb[c], op_, qf_col[:, c:c + 1])
            o2scr = sb2.tile([P, P], bf16, tag="junk", name="o2scr")
            nc.scalar.activation(o2scr, op_, AF.Square, scale=qf_col[:, c:c + 1],
                                 accum_out=ssq_sb[:, c:c + 1])
            # S update: S[i] = lam*S[i] + Knat[i].T @ W2
            sup = ps2.tile([P, 4 * P], fp32, tag="big", name="sup", bufs=3)
            for i in range(NDT):
                nc.tensor.matmul(sup[:, i * P:(i + 1) * P],
                                 Knat[c][:, i * P:(i + 1) * P], W2,
                                 start=True, stop=True)
            newS = sing.tile([P, NDT, P], bf16, tag=f"St_{c % 2}",
                             name=f"St_{c % 2}")
            nc.vector.scalar_tensor_tensor(
                newS, S_t, lamB[:, c:c + 1],
                sup.rearrange("p (i t) -> p i t", i=NDT), OP.mult, OP.add)
            S_t = newS
            # gate projection + silu for this chunk (fills PE, keeps HAM warm)
            gp = ps2.tile([P, P], fp32, tag="pair", name="gp")
            for kt in range(NDT):
                nc.tensor.matmul(gp, xT3[:, kt, c * P:(c + 1) * P],
                                 wg_sb[:, kt * P:(kt + 1) * P],
                                 start=(kt == 0), stop=(kt == NDT - 1))
            g_silu = sb2.tile([P, P], fp32, tag="gsil", name="g_silu")
            if USE_SILU_TABLE:
                act_order.append(nc.scalar.activation(g_silu, gp, AF.Silu))
            else:
                gsg = sb2.tile([P, P], fp32, tag="gsg", name="gsg")
                act_order.append(nc.scalar.activation(gsg, gp, AF.Sigmoid))
                nc.vector.tensor_mul(g_silu, gp, gsg)
            # partial output projection for this chunk (rsq post-RS)
            t1 = sb2.tile([P, P], fp32, tag="t1", name="t1")
            nc.vector.tensor_mul(t1, o_sb[c], wnormB)
            t3 = sb2.tile([P, P], bf16, tag="t3", name="t3")
            nc.vector.tensor_mul(t3, t1, g_silu)
            ttp = ps2.tile([P, P], bf16, tag="pair", name="t3p")
            nc.tensor.transpose(ttp, t3, ident_bf)
            t3T = sb2.tile([P, P], bf16, tag="t3T", name="t3T")
            nc.vector.tensor_copy(t3T, ttp)
            po = ps2.tile([P, 512], fp32, tag="big", name="po", bufs=3)
            nc.tensor.matmul(po, t3T, wo_sb, start=True, stop=True)
            pob = sb2.tile([P, 520], bf16, tag="pob", name="pob")
            nc.vector.tensor_copy(pob[:, 0:512], po)
            nc.vector.tensor_copy(pob[:, 512:513], ssq_sb[:, c:c + 1])
            nc.vector.memset(pob[:, 513:520], 0.0)
            nc.sync.dma_start(part[c * P:(c + 1) * P, :], pob)

        # ── ReduceScatter partials (with packed ssq col): row-quarters ──
        rs_out = dramp.tile([2 * P, 520], bf16, name="rs_out")
        nc.gpsimd.collective_compute(
            kind="ReduceScatter", op=OP.add,
            replica_groups=[[0, 1, 2, 3], [4, 5, 6, 7]],
            ins=[part[:]], outs=[rs_out[:]])
        rs_sb = sing.tile([P, 2, 520], bf16)
        nc.sync.dma_start(rs_sb, rs_out.rearrange("(a p) d -> p a d", p=P))
        # rsq = exp(-0.5 ln(ssq/D + 1e-5)) applied to the reduced rows
        lnsq = sing.tile([P, 2], fp32)
        nc.scalar.activation(lnsq, rs_sb[:, :, 512], AF.Ln,
                             bias=eps5c[:, 0:1], scale=1.0 / D)
        rsq2 = sing.tile([P, 2], fp32)
        nc.scalar.activation(rsq2, lnsq, AF.Exp, scale=-0.5)
        orows = sing.tile([P, 2, 512], bf16)
        for a_ in range(2):
            nc.vector.tensor_scalar_mul(orows[:, a_], rs_sb[:, a_, 0:512],
                                        rsq2[:, a_:a_ + 1])
        nc.sync.dma_start(out_rows.rearrange("(a p) d -> p a d", p=P), orows)


# ─────────────────────────────── L2 kernel ────────────────────────────────
def l2_kernel(tc, ins, outs):
    import concourse.mybir as mybir
    nc = tc.nc
    fp32 = mybir.dt.float32
    bf16 = mybir.dt.bfloat16
    AF = mybir.ActivationFunctionType
    OP = mybir.AluOpType
    TQ = 256  # rows per core

    oT, ssqr, xTs, wgT, woT, wnorm = (ins[k] for k in
                                      ("oT", "ssqr", "xTs", "wgT", "woT", "wnorm"))
    out_rows = outs["out"]

    ctx = ExitStack()
    with ctx:
        sing = ctx.enter_context(tc.tile_pool(name="sing", bufs=1))
        sb2 = ctx.enter_context(tc.tile_pool(name="sb2", bufs=2))
        ps2 = ctx.enter_context(tc.tile_pool(name="ps2", bufs=2, space="PSUM"))

        oT_sb = sing.tile([P, 4 * TQ], bf16)
        for i in range(2):
            nc.sync.dma_start(oT_sb[:, i * 2 * TQ:(i + 1) * 2 * TQ],
                              oT[:, i * 2 * TQ:(i + 1) * 2 * TQ])
        xT_sb = sing.tile([P, 4 * TQ], bf16)
        nc.sync.dma_start(xT_sb, xTs)
        wg_sb = sing.tile([P, 4 * 512], bf16)
        for i in range(2):
            nc.sync.dma_start(wg_sb[:, i * 1024:(i + 1) * 1024],
                              wgT[:, i * 1024:(i + 1) * 1024])
        wo_sb = sing.tile([P, 4 * 512], bf16)
        for i in range(2):
            nc.sync.dma_start(wo_sb[:, i * 1024:(i + 1) * 1024],
                              woT[:, i * 1024:(i + 1) * 1024])
        wn_sb = sing.tile([P, 4], fp32)
        nc.sync.dma_start(wn_sb, wnorm)
        ssq_sb = sing.tile([1, TQ], fp32)
        nc.sync.dma_start(ssq_sb, ssqr)
        eps5 = sing.tile([1, 1], fp32)
        nc.vector.memset(eps5, 1e-5)

        # rsq row -> broadcast
        srt = sing.tile([1, TQ], fp32)
        nc.scalar.activation(srt, ssq_sb, AF.Sqrt, bias=eps5[:, 0:1], scale=1.0 / D)
        rsq = sing.tile([1, TQ], fp32)
        nc.vector.reciprocal(rsq, srt)
        rsqB = sing.tile([P, TQ], fp32)
        nc.gpsimd.partition_broadcast(rsqB, rsq)

        # gateT = Wg^T-proj of x slice; t3 = o*rsq*wnorm*silu(gate), bf16
        t3 = [sing.tile([P, TQ], bf16, tag=f"t3_{i}", name=f"t3_{i}") for i in range(NDT)]
        for i in range(NDT):
            gp = ps2.tile([P, TQ], fp32, tag="gp")
            for kt in range(NDT):
                nc.tensor.matmul(gp, wg_sb[:, kt * 512 + i * P: kt * 512 + (i + 1) * P],
                                 xT_sb[:, kt * TQ:(kt + 1) * TQ],
                                 start=(kt == 0), stop=(kt == NDT - 1))
            sgs = sb2.tile([P, TQ], fp32, tag="sgs", name="sgs")
            nc.scalar.activation(sgs, gp, AF.Sigmoid)
            sg = sb2.tile([P, TQ], fp32, tag="sg", name="sg")
            nc.vector.tensor_mul(sg, gp, sgs)
            t1 = sb2.tile([P, TQ], fp32, tag="t1")
            nc.vector.scalar_tensor_tensor(t1, oT_sb[:, i * TQ:(i + 1) * TQ],
                                           wn_sb[:, i:i + 1], rsqB,
                                           OP.mult, OP.mult)
            nc.vector.tensor_mul(t3[i], t1, sg)

        for tt in range(TQ // P):
            pp = ps2.tile([P, 512], fp32, tag="pp")
            for i in range(NDT):
                nc.tensor.matmul(pp, t3[i][:, tt * P:(tt + 1) * P],
                                 wo_sb[:, i * 512:(i + 1) * 512],
                                 start=(i == 0), stop=(i == NDT - 1))
            ob = sb2.tile([P, 512], fp32, tag="ob")
            nc.vector.tensor_copy(ob, pp)
            nc.sync.dma_start(out_rows[tt * P:(tt + 1) * P, :], ob)


# ───────────────────────────── host-side prep ─────────────────────────────
def _tile512(a):
    # [512, N] -> [128, 4*N] with col = kt*N + j
    n = a.shape[1]
    return np.ascontiguousarray(
        a.reshape(NDT, P, n).transpose(1, 0, 2).reshape(P, NDT * n))


def prep_l1(x, q_proj_w, k_proj_w, v_proj_w, b_proj_w, a_proj_w, A_log,
            dt_bias, q_conv_w, k_conv_w, v_conv_w, g_proj_w=None,
            o_norm_w=None, o_proj_w=None):
    wqk = _tile512(np.concatenate([q_proj_w.T, k_proj_w.T], 1)).astype(BF16)
    sc = np.zeros((1, 8), np.float32)
    sc[0, 0] = float(dt_bias[0])
    sc[0, 1] = -float(np.exp(A_log[0]))
    xTs = [_tile512(np.ascontiguousarray(x[b].T)).astype(BF16) for b in range(B)]
    ins = []
    for b in range(B):
        for s in range(NDT):
            vsl = slice(s * P, (s + 1) * P)
            convw = np.zeros((P, 36), np.float32)
            for i in range(NDT):
                convw[:, 4 * i:4 * (i + 1)] = q_conv_w[i * P:(i + 1) * P]
                convw[:, 16 + 4 * i:16 + 4 * (i + 1)] = k_conv_w[i * P:(i + 1) * P]
            convw[:, 32:36] = v_conv_w[vsl]
            ins.append({
                "xT": xTs[b],
                "wqk": wqk,
                "wv": _tile512(np.ascontiguousarray(v_proj_w.T[:, vsl])).astype(BF16),
                "wba": _tile512(np.concatenate([b_proj_w.T, a_proj_w.T], 1)).astype(BF16),
                "convw": convw,
                "sc": sc,
                "wgs": _tile512(np.ascontiguousarray(g_proj_w.T[:, vsl])).astype(BF16),
                "wos": np.ascontiguousarray(o_proj_w.T[vsl, :]).astype(BF16),
                "wnr": np.ascontiguousarray(o_norm_w[vsl]).reshape(1, P).astype(np.float32),
            })
    return ins


def prep_l2(l1_results, x, g_proj_w, o_norm_w, o_proj_w):
    # assemble o [B, 1024, 512] and ssq [B, 1024]
    o = np.zeros((B, T, D), np.float32)
    ssq = np.zeros((B, T), np.float32)
    for b in range(B):
        for s in range(NDT):
            r = l1_results[b * NDT + s]
            o[b, :, s * P:(s + 1) * P] = r["o"].transpose(1, 0, 2).reshape(T, P)
            ssq[b] += r["ssq"].T.reshape(T)
    wgT = _tile512(np.ascontiguousarray(g_proj_w.T)).astype(BF16)
    woT = _tile512(np.ascontiguousarray(o_proj_w.T)).astype(BF16)
    wnorm = np.ascontiguousarray(o_norm_w.reshape(NDT, P).T).astype(np.float32)
    TQ = 256
    ins = []
    for b in range(B):
        oTb = np.ascontiguousarray(o[b].T)          # [512, 1024]
        xTb = np.ascontiguousarray(x[b].T)
        for q in range(4):
            tsl = slice(q * TQ, (q + 1) * TQ)
            ins.append({
                "oT": _tile512(np.ascontiguousarray(oTb[:, tsl])).astype(BF16),
                "ssqr": np.ascontiguousarray(ssq[b, tsl]).reshape(1, TQ),
                "xTs": _tile512(np.ascontiguousarray(xTb[:, tsl])).astype(BF16),
                "wgT": wgT, "woT": woT, "wnorm": wnorm,
            })
    return ins


# ─────────────────────────── build + run (spmd) ───────────────────────────
def _build(kern, in_specs, out_specs):
    import concourse.mybir as mybir
    import concourse.tile as tile
    from concourse import bacc
    nc = bacc.Bacc(None, target_bir_lowering=False)
    with tile.TileContext(nc) as tc:
        with tc.tile_pool(name="io", bufs=1, space="DRAM") as io:
            ins = {k: io.tile(shape, dt, kind="ExternalInput", name=f"in_{k}")
                   for k, (shape, dt) in in_specs.items()}
            outs = {k: io.tile(shape, dt, kind="ExternalOutput", name=f"out_{k}")
                    for k, (shape, dt) in out_specs.items()}
            kern(tc, {k: v[:] for k, v in ins.items()},
                 {k: v[:] for k, v in outs.items()})
    nc.compile()
    return nc, ins, outs


_CACHE = {}


def _specs_l1():
    import concourse.mybir as mybir
    f, h = mybir.dt.float32, mybir.dt.bfloat16
    in_specs = {"xT": ((P, NDT * 1024), h), "wqk": ((P, NDT * 1024), h),
                "wv": ((P, NDT * P), h), "wba": ((P, NDT * 2), h),
                "convw": ((P, 36), f), "sc": ((1, 8), f),
                "wgs": ((P, NDT * P), h), "wos": ((P, 512), h),
                "wnr": ((1, P), f)}
    out_specs = {"out": ((2 * P, 512), h)}
    return in_specs, out_specs


def _specs_l2():
    import concourse.mybir as mybir
    f, h = mybir.dt.float32, mybir.dt.bfloat16
    TQ = 256
    in_specs = {"oT": ((P, NDT * TQ), h), "ssqr": ((1, TQ), f),
                "xTs": ((P, NDT * TQ), h), "wgT": ((P, NDT * 512), h),
                "woT": ((P, NDT * 512), h), "wnorm": ((P, NDT), f)}
    out_specs = {"out": ((TQ, 512), f)}
    return in_specs, out_specs


def run_spmd(which, kern, specs, in_dicts, trace):
    from concourse.bass_utils import run_bass_kernel_spmd
    install_ntff_shim()
    if which not in _CACHE:
        _CACHE[which] = _build(kern, *specs)
    nc, ins, outs = _CACHE[which]
    in_maps = [{ins[k].name: np.ascontiguousarray(v) for k, v in d.items()}
               for d in in_dicts]
    t0 = time.perf_counter()
    try:
        res = run_bass_kernel_spmd(nc, in_maps, list(range(len(in_dicts))),
                                   trace=trace)
    except Exception:
        if not trace:
            raise
        res = run_bass_kernel_spmd(nc, in_maps, list(range(len(in_dicts))),
                                   trace=False)
    wall_ns = int((time.perf_counter() - t0) * 1e9)
    outl = [{k: np.asarray(res.results[c][outs[k].name])
             for k in outs} for c in range(len(in_dicts))]
    return outl, (res.exec_time_ns if res.exec_time_ns else wall_ns)


def kernel(x, q_proj_w, k_proj_w, v_proj_w, b_proj_w, a_proj_w, A_log,
           dt_bias, q_conv_w, k_conv_w, v_conv_w, g_proj_w, o_norm_w,
           o_proj_w, trace=True):
    args = [np.asarray(a, np.float32) for a in
            (x, q_proj_w, k_proj_w, v_proj_w, b_proj_w, a_proj_w, A_log,
             dt_bias, q_conv_w, k_conv_w, v_conv_w, g_proj_w, o_norm_w,
             o_proj_w)]
    (x, q_proj_w, k_proj_w, v_proj_w, b_proj_w, a_proj_w, A_log, dt_bias,
     q_conv_w, k_conv_w, v_conv_w, g_proj_w, o_norm_w, o_proj_w) = args

    ins1 = prep_l1(x, q_proj_w, k_proj_w, v_proj_w, b_proj_w, a_proj_w,
                   A_log, dt_bias, q_conv_w, k_conv_w, v_conv_w,
                   g_proj_w, o_norm_w, o_proj_w)
    r1, ns1 = run_spmd("l1", l1_kernel, _specs_l1(), ins1, trace)
    out = np.zeros((B, T, D), np.float32)
    for b in range(B):
        for s in range(4):
            out[b, s * 256:(s + 1) * 256] = r1[b * 4 + s]["out"].astype(np.float32)
    _LAST_HW_NS[0] = ns1
    return out



# revision 20
# speedup vs baseline: 1.3800x; 1.3800x over previous
"""GatedDeltaNet (B=2, T=1024, D=512, H=1) fully on-device on 8 trn2 cores.

Sharding: core (b, s) = batch b x Dv-slice s (128 v-columns of the state).
The sequential delta-rule scan parallelizes over Dv with no cross-core
traffic; the final cross-slice combine is a small AllToAll of the raw
pre-norm o slices (256KB/core) instead of a ReduceScatter of projected
partials (1MB/core); each core then computes RMSNorm + gate + output
projection for its own 256-row t-quarter.

Single launch per 8 cores:
  * bf16 projections (q,k full + v slice) via PE, causal dwconv as
    diagonal-matmuls on PE, silu via the HW act table.
  * beta/a projected directly in column space ([t-part, chunk] layout)
    with 2-col RHS matmuls; in-chunk cumsum of g as a lower-triangular
    ones matmul; l2norm scales via Sqrt+reciprocal (no fp32 1-col
    transpose matmuls, minimal act-table switching).
  * chunked (C=128) gated delta rule with the exact log-depth Neumann
    factorization truncated at (I-M)(I+M^2); bf16 operands, f32 PSUM.
  * o chunks stream to DRAM during the scan; one 4-core AllToAll hands
    each core the full-Dh o rows for its t-quarter; tail computes
    RMSNorm, silu(gate) and the output projection on-device.
HW exec time is the genuine NTFF-profiled device execution time
(max traced core), captured via the axon NRT profile hook.
"""

import sys
import types
import time
from contextlib import ExitStack

import numpy as np
import ml_dtypes

BF16 = ml_dtypes.bfloat16

B, T, D, KC = 2, 1024, 512, 4
P = 128
C = 128          # chunk length
NCH = T // C     # 8 chunks
NDT = D // P     # 4 dk tiles

_LAST_HW_NS = [None]
USE_SILU_TABLE = True   # real HW has a silu table; CoreSim does not


# ─────────────────────────── axon NTFF hook shim ──────────────────────────
def install_ntff_shim():
    if "antenv.axon_hooks" in sys.modules:
        return
    try:
        import antenv  # noqa: F401
        from trn_agent_boot.trn_boot import _ntff_profile_via_ctypes
        hook = _ntff_profile_via_ctypes('/opt/axon/libaxon_pjrt.so')
    except Exception:
        hook = None
    mod = types.ModuleType("antenv.axon_hooks")
    mod.get_axon_ntff_profile_hook = lambda: hook
    mod.set_axon_ntff_profile_hook = lambda h: None
    sys.modules["antenv.axon_hooks"] = mod


# ─────────────────────────────── L1 kernel ────────────────────────────────
def l1_kernel(tc, ins, outs):
    import concourse.bass as bass
    import concourse.mybir as mybir
    from concourse.masks import make_identity

    nc = tc.nc
    fp32 = mybir.dt.float32
    bf16 = mybir.dt.bfloat16
    AF = mybir.ActivationFunctionType
    OP = mybir.AluOpType

    xT, wqk, wv, wba, convw, sc = (ins[k] for k in
                                   ("xT", "wqk", "wv", "wba", "convw", "sc"))
    wg, wo, wnr, xq, bmask = (ins[k] for k in ("wg", "wo", "wnr", "xq", "bmask"))
    out_rows = outs["out"]

    ctx = ExitStack()
    with ctx:
        sing = ctx.enter_context(tc.tile_pool(name="sing", bufs=1))
        sb2 = ctx.enter_context(tc.tile_pool(name="sb2", bufs=2))
        ps = ctx.enter_context(tc.tile_pool(name="ps", bufs=1, space="PSUM"))
        ps2 = ctx.enter_context(tc.tile_pool(name="ps2", bufs=2, space="PSUM"))
        dramp = ctx.enter_context(tc.tile_pool(name="dramp", bufs=1, space="DRAM"))

        # ── load inputs (two pieces per big tensor) ──
        xT_sb = sing.tile([P, NDT * 1024], bf16)
        xT_v = xT.rearrange("p (k t) -> p k t", k=NDT)
        xTs_v = xT_sb.rearrange("p (k t) -> p k t", k=NDT)
        for kt, eng_ in ((0, nc.sync), (1, nc.scalar), (2, nc.gpsimd),
                         (3, nc.sync)):
            eng_.dma_start(xTs_v[:, kt], xT_v[:, kt])
        wqk_sb = sing.tile([P, NDT * 1024], bf16)
        wqk_v = wqk.rearrange("p (k t) -> p k t", k=NDT)
        wqks_v = wqk_sb.rearrange("p (k t) -> p k t", k=NDT)
        for th, eng_ in ((0, nc.gpsimd), (1, nc.scalar)):
            eng_.dma_start(wqks_v[:, :, th * 512:(th + 1) * 512],
                           wqk_v[:, :, th * 512:(th + 1) * 512])
        wv_sb = sing.tile([P, NDT * P], bf16)
        nc.sync.dma_start(wv_sb, wv)
        wg_sb = sing.tile([P, NDT * 512], bf16)
        nc.scalar.dma_start(wg_sb, wg)
        wo_sb = sing.tile([P, NDT * 512], bf16)
        nc.gpsimd.dma_start(wo_sb, wo)
        xq_sb = sing.tile([P, NDT * 256], bf16)
        nc.sync.dma_start(xq_sb, xq)
        wnr_sb = sing.tile([1, 512], fp32)
        nc.sync.dma_start(wnr_sb, wnr)
        wnormF = sing.tile([P, 512], fp32)
        nc.gpsimd.partition_broadcast(wnormF, wnr_sb)
        wba_sb = sing.tile([P, NDT * 2], bf16)
        nc.sync.dma_start(wba_sb, wba)
        convw_sb = sing.tile([P, 36], fp32)
        nc.sync.dma_start(convw_sb, convw)
        sc_sb = sing.tile([1, 8], fp32)
        nc.sync.dma_start(sc_sb, sc)
        scB = sing.tile([P, 8], fp32)
        nc.gpsimd.partition_broadcast(scB, sc_sb)
        bm_sb = sing.tile([1, 2], fp32)
        nc.sync.dma_start(bm_sb, bmask)
        bmB = sing.tile([P, 2], fp32)
        nc.gpsimd.partition_broadcast(bmB, bm_sb)

        ident = sing.tile([P, P], fp32)
        make_identity(nc, ident)
        ones128 = sing.tile([P, P], fp32)
        nc.vector.memset(ones128, 1.0)
        epsc = sing.tile([P, 1], fp32)
        nc.vector.memset(epsc, 1e-6)
        epsDc = sing.tile([P, 1], fp32)
        nc.vector.memset(epsDc, float(D) * 1e-6)
        eps5c = sing.tile([P, 1], fp32)
        nc.vector.memset(eps5c, 1e-5)
        # one-hot row 127 selector
        oh127 = sing.tile([P, 1], fp32)
        nc.vector.memset(oh127, 1.0)
        nc.gpsimd.affine_select(oh127, oh127, [[0, 1]], OP.is_equal, 0.0,
                                base=-127, channel_multiplier=1)
        # lower-triangular (incl diag) ones, fp32: keep where f >= p
        trilf = sing.tile([P, P], fp32)
        nc.vector.memset(trilf, 1.0)
        nc.gpsimd.affine_select(trilf, trilf, [[1, P]], OP.is_ge, 0.0,
                                base=0, channel_multiplier=-1)

        xT3 = xT_sb.rearrange("p (k t) -> p k t", k=NDT)
        wba3 = wba_sb.rearrange("p (k t) -> p k t", k=NDT)
        wg3 = wg_sb.rearrange("p (k t) -> p k t", k=NDT)
        wo3 = wo_sb.rearrange("p (k t) -> p k t", k=NDT)
        xq3 = xq_sb.rearrange("p (k t) -> p k t", k=NDT)

        # ── beta/a projections straight into column space [t-part, chunk] ──
        bac = sing.tile([P, NCH, 2], fp32)
        for c in range(NCH):
            bap = ps2.tile([P, 2], fp32, tag="pair", name="bap")
            for kt in range(NDT):
                nc.tensor.matmul(bap, xT3[:, kt, c * P:(c + 1) * P],
                                 wba3[:, kt, :],
                                 start=(kt == 0), stop=(kt == NDT - 1))
            nc.vector.tensor_copy(bac[:, c], bap)
        bl_col = bac[:, :, 0]
        a_col = bac[:, :, 1]

        # g = -exp(A_log) * softplus(a + dt_bias) = ln(1+exp(a+bias)), col space
        onec = sing.tile([P, 1], fp32)
        nc.vector.memset(onec, 1.0)
        spe_col = sing.tile([P, NCH], fp32)
        nc.scalar.activation(spe_col, a_col, AF.Exp,
                             bias=scB[:, 0:1], scale=1.0)
        sp_col = sing.tile([P, NCH], fp32)
        nc.scalar.activation(sp_col, spe_col, AF.Ln, bias=onec[:, 0:1])
        g_col = sing.tile([P, NCH], fp32)
        nc.vector.tensor_scalar_mul(g_col, sp_col, scB[:, 1:2])
        # L = in-chunk inclusive cumsum of g  (tril ones matmul, fp32)
        Lp = ps2.tile([P, NCH], fp32, tag="pair", name="Lp")
        nc.tensor.matmul(Lp, trilf, g_col, start=True, stop=True)
        L_col = sing.tile([P, NCH], fp32)
        nc.vector.tensor_copy(L_col, Lp)

        # beta = sigmoid(bl) via exp + reciprocal (Exp table)
        bexp = sing.tile([P, NCH], fp32)
        nc.scalar.activation(bexp, bl_col, AF.Exp, scale=-1.0)
        bexp1 = sing.tile([P, NCH], fp32)
        nc.vector.tensor_scalar_add(bexp1, bexp, 1.0)
        bcol = sing.tile([P, NCH], fp32)
        nc.vector.reciprocal(bcol, bexp1)
        lam_col = sing.tile([P, NCH], fp32)
        nc.scalar.activation(lam_col, L_col, AF.Exp)
        eneg_col = sing.tile([P, NCH], fp32)
        nc.scalar.activation(eneg_col, L_col, AF.Exp, scale=-1.0)
        # lam per chunk end: lam8row = oh127^T @ lam_col, bcast down partitions
        l8p = ps2.tile([1, NCH], fp32, tag="pair", name="l8p")
        nc.tensor.matmul(l8p, oh127, lam_col, start=True, stop=True)
        lam8 = sing.tile([1, NCH], fp32)
        nc.vector.tensor_copy(lam8, l8p)
        lamB = sing.tile([P, NCH], fp32)
        nc.gpsimd.partition_broadcast(lamB, lam8)

        # conv-as-matmul: per (group, tap) diagonal weight tiles
        ident_bf = sing.tile([P, P], bf16)
        nc.vector.tensor_copy(ident_bf, ident)
        diag4 = [sing.tile([P, 4, P], bf16, tag=f"diag4_{g}", name=f"diag4_{g}")
                 for g in range(9)]
        for g in range(9):
            for j in range(4):
                nc.vector.tensor_scalar_mul(diag4[g][:, j, :], ident_bf,
                                            convw_sb[:, 4 * g + j:4 * g + j + 1])

        # ── projections + conv pads ──
        pads = {}
        for name, n_dt in (("q", NDT), ("k", NDT), ("v", 1)):
            for dt_i in range(n_dt):
                pad = sing.tile([P, 3 + 1024], bf16, tag=f"pad_{name}{dt_i}",
                                name=f"pad_{name}{dt_i}")
                nc.vector.memset(pad[:, 0:3], 0.0)
                pads[(name, dt_i)] = pad

        wqk3 = wqk_sb.rearrange("p (k t) -> p k t", k=NDT)
        wv3 = wv_sb.rearrange("p (k t) -> p k t", k=NDT)
        ecnt = 0
        for th in range(2):  # t-half, 512 cols
            for name, n_dt in (("q", NDT), ("k", NDT), ("v", 1)):
                for dt_i in range(n_dt):
                    pp = ps2.tile([P, 512], fp32, tag="big", name="pp", bufs=3)
                    for kt in range(NDT):
                        if name == "q":
                            lhs_ = wqk3[:, kt, dt_i * P:(dt_i + 1) * P]
                        elif name == "k":
                            lhs_ = wqk3[:, kt, 512 + dt_i * P: 512 + (dt_i + 1) * P]
                        else:
                            lhs_ = wv3[:, kt]
                        nc.tensor.matmul(
                            pp, lhs_, xT3[:, kt, th * 512:(th + 1) * 512],
                            start=(kt == 0), stop=(kt == NDT - 1))
                    if ecnt % 4 == 0:
                        nc.scalar.copy(
                            pads[(name, dt_i)][:, 3 + th * 512: 3 + (th + 1) * 512], pp)
                    else:
                        nc.vector.tensor_copy(
                            pads[(name, dt_i)][:, 3 + th * 512: 3 + (th + 1) * 512], pp)
                    ecnt += 1

        # conv (4 taps) + silu (scalar act table).
        # kq_all[p, i, c, 0:128]=K-chunk, [...,128:256]=Q-chunk
        kq_all = sing.tile([P, NDT, NCH, 2 * P], bf16)
        kqcat = [kq_all[:, i] for i in range(NDT)]
        vTf = sing.tile([P, 1024], bf16)

        def kslice(i, c):
            return kq_all[:, i, c, 0:P]

        grp_idx = {("q", 0): 0, ("q", 1): 1, ("q", 2): 2, ("q", 3): 3,
                   ("k", 0): 4, ("k", 1): 5, ("k", 2): 6, ("k", 3): 7,
                   ("v", 0): 8}
        for (name, dt_i), pad in pads.items():
            g = grp_idx[(name, dt_i)]
            for th in range(2):
                cps = ps2.tile([P, 512], fp32, tag="big", name="cps", bufs=3)
                for j in range(4):
                    nc.tensor.matmul(cps, diag4[g][:, j],
                                     pad[:, th * 512 + j: th * 512 + j + 512],
                                     start=(j == 0), stop=(j == 3))
                cv = cps.rearrange("p (c t) -> p c t", c=4)
                crange = slice(th * 4, (th + 1) * 4)
                if USE_SILU_TABLE:
                    if name == "v":
                        nc.scalar.activation(
                            vTf[:, th * 512:(th + 1) * 512], cps, AF.Silu)
                    else:
                        off = 0 if name == "k" else P
                        nc.scalar.activation(
                            kqcat[dt_i][:, crange, off:off + P], cv, AF.Silu)
                else:
                    sg = sb2.tile([P, 512], bf16, tag="sgc", name="sgc")
                    nc.scalar.activation(sg, cps, AF.Sigmoid)
                    sgv = sg.rearrange("p (c t) -> p c t", c=4)
                    if name == "v":
                        nc.vector.tensor_mul(vTf[:, th * 512:(th + 1) * 512],
                                             cps, sg)
                    else:
                        off = 0 if name == "k" else P
                        nc.vector.tensor_mul(kqcat[dt_i][:, crange, off:off + P],
                                             cv, sgv)

        # ── chunk pass A: KK/KQ (stored) + QQ; diag -> sumsq cols ──
        kkq_sb = [sing.tile([P, 2 * P], bf16, tag=f"kkq{c}", name=f"kkq{c}")
                  for c in range(NCH)]
        ssqk_col = sing.tile([P, NCH], fp32)
        ssqq_col = sing.tile([P, NCH], fp32)
        for c in range(NCH):
            kkq = ps2.tile([P, 3 * P], fp32, tag="big", name="kkq", bufs=3)
            for i in range(NDT):
                nc.tensor.matmul(kkq[:, 0:2 * P], kq_all[:, i, c, 0:P],
                                 kq_all[:, i, c, :],
                                 start=(i == 0), stop=(i == NDT - 1))
            for i in range(NDT):
                nc.tensor.matmul(kkq[:, 2 * P:3 * P], kq_all[:, i, c, P:2 * P],
                                 kq_all[:, i, c, P:2 * P],
                                 start=(i == 0), stop=(i == NDT - 1))
            nc.vector.tensor_copy(kkq_sb[c], kkq[:, 0:2 * P])
            junk = sb2.tile([P, P], bf16, tag="junk", name="junk")
            nc.vector.scalar_tensor_tensor(junk, kkq_sb[c][:, 0:P], 1.0, ident,
                                           OP.mult, OP.mult,
                                           accum_out=ssqk_col[:, c:c + 1])
            junk2 = sb2.tile([P, P], bf16, tag="junk", name="junk2")
            nc.vector.scalar_tensor_tensor(junk2, kkq[:, 2 * P:3 * P], 1.0, ident,
                                           OP.mult, OP.mult,
                                           accum_out=ssqq_col[:, c:c + 1])

        # ── l2norm scales via Sqrt + reciprocal (Sqrt table) ──
        srk = sing.tile([P, NCH], fp32)
        nc.scalar.activation(srk, ssqk_col, AF.Sqrt, bias=epsc[:, 0:1])
        rk_col = sing.tile([P, NCH], fp32)
        nc.vector.reciprocal(rk_col, srk)
        srq = sing.tile([P, NCH], fp32)
        nc.scalar.activation(srq, ssqq_col, AF.Sqrt, bias=epsDc[:, 0:1],
                             scale=float(D))
        rq_col = sing.tile([P, NCH], fp32)
        nc.vector.reciprocal(rq_col, srq)

        colf_col = sing.tile([P, NCH], fp32)
        nc.vector.tensor_mul(colf_col, eneg_col, rk_col)
        qf_col = sing.tile([P, NCH], fp32)
        nc.vector.tensor_mul(qf_col, lam_col, rq_col)
        rowfM_col = sing.tile([P, NCH], fp32)
        nc.vector.tensor_mul(rowfM_col, bcol, lam_col)
        nc.vector.tensor_mul(rowfM_col, rowfM_col, rk_col)
        rowfMn_col = sing.tile([P, NCH], fp32)
        nc.vector.tensor_scalar_mul(rowfMn_col, rowfM_col, -1.0)
        kbar_col = sing.tile([P, NCH], fp32)
        nc.vector.tensor_mul(kbar_col, colf_col, lamB)

        # colf as per-free chunk tiles: pack -> PE transpose -> broadcasts
        colf_bf = sing.tile([P, NCH], bf16)
        nc.vector.tensor_copy(colf_bf, colf_col)
        cfp = ps2.tile([NCH, P], bf16, tag="pair", name="cfp")
        nc.tensor.transpose(cfp, colf_bf, ident_bf)
        colfT = sing.tile([NCH, P], bf16)
        nc.vector.tensor_copy(colfT, cfp)
        colfDr = dramp.tile([NCH, P], bf16, name="colfDr")
        nc.sync.dma_start(colfDr, colfT)
        colfRow = sing.tile([1, NCH * P], bf16)
        nc.sync.dma_start(colfRow, colfDr.rearrange("c d -> (c d)"))
        colfBall = sing.tile([P, NCH * P], bf16)
        nc.gpsimd.partition_broadcast(colfBall, colfRow)
        colfB = [colfBall[:, c * P:(c + 1) * P] for c in range(NCH)]

        # ── V transpose + beta scale ──
        Vb = [sing.tile([P, P], bf16, tag=f"Vb{c}", name=f"Vb{c}")
              for c in range(NCH)]
        for c in range(NCH):
            tp = ps2.tile([P, 2 * P], bf16, tag="pair", name="vtp")[:, 0:P]
            nc.tensor.transpose(tp, vTf[:, c * C:(c + 1) * C], ident_bf)
            nc.vector.tensor_scalar_mul(Vb[c], tp, bcol[:, c:c + 1])

        # ── K natural-layout [t, d] chunks (for the state update) ──
        Knat = [sing.tile([P, D], bf16, tag=f"Knat{c}", name=f"Knat{c}")
                for c in range(NCH)]
        for c in range(NCH):
            for half in range(2):
                tpk = ps2.tile([P, 2 * P], bf16, tag="pair", name="tpk")
                nc.tensor.transpose(tpk[:, 0:P], kslice(2 * half, c), ident_bf)
                nc.tensor.transpose(tpk[:, P:2 * P], kslice(2 * half + 1, c), ident_bf)
                nc.vector.tensor_copy(Knat[c][:, half * 2 * P:(half + 1) * 2 * P], tpk)

        # ── chunk pass B: M, MT, M2 pair, TT, AmatT ──
        AmatT = [sing.tile([P, P], bf16, tag=f"Am{c}", name=f"Am{c}")
                 for c in range(NCH)]
        TTs = [sing.tile([P, P], bf16, tag=f"TT{c}", name=f"TT{c}")
               for c in range(NCH)]
        for c in range(NCH):
            # M [t,i] strict lower
            M = sb2.tile([P, P], bf16, tag="M", name="M")
            nc.vector.scalar_tensor_tensor(M, kkq_sb[c][:, 0:P],
                                           rowfM_col[:, c:c + 1],
                                           colfB[c], OP.mult, OP.mult)
            nc.gpsimd.affine_select(M, M, [[-1, P]], OP.is_ge, 0.0,
                                    base=-1, channel_multiplier=1)
            # MT via PE transpose (already masked)
            mtp = ps2.tile([P, 2 * P], bf16, tag="pair", name="cm")[:, 0:P]
            nc.tensor.transpose(mtp, M, ident_bf)
            MT = sb2.tile([P, P], bf16, tag="MT", name="MT")
            nc.vector.tensor_copy(MT, mtp)
            # AmatT [i,t] upper incl diag
            nc.vector.tensor_scalar_mul(AmatT[c], kkq_sb[c][:, P:2 * P],
                                        colf_col[:, c:c + 1])
            nc.gpsimd.affine_select(AmatT[c], AmatT[c], [[1, P]], OP.is_ge,
                                    0.0, base=0, channel_multiplier=-1)
            # M2
            pr = ps2.tile([P, 2 * P], fp32, tag="pair", name="pair")
            nc.tensor.matmul(pr[:, 0:P], MT, M, start=True, stop=True)
            pw0 = sb2.tile([P, P], bf16, tag="pw0", name="pw0")
            nc.vector.tensor_copy(pw0, pr[:, 0:P])
            # TT = (I + M2T)(I - MT)
            tt = sb2.tile([P, P], bf16, tag="ttp", name="ttp")
            nc.vector.tensor_sub(tt, ident, MT)
            tp_ = ps2.tile([P, P], fp32, tag="pair", name="ttps")
            nc.tensor.matmul(tp_, pw0, tt, start=True, stop=True)
            nc.vector.tensor_add(TTs[c], tt, tp_)

        # ── serial scan; o chunks stream to DRAM for the AllToAll ──
        # 8-core AllToAll per t-half: block j of call h carries my o chunk
        # (4h + j%4); each core r=(b,s) receives, in row-block i, core i's
        # chunk (4h+s) = global t rows [512h+128s : +128] for dv-slice i%4.
        # Cross-batch blocks (i//4 != b) are discarded with the bmask merge.
        S_t = sing.tile([P, NDT, P], bf16, tag="St_1", name="St_init")
        nc.vector.memset(S_t, 0.0)
        a2a_in = [dramp.tile([T, P], bf16, name=f"a2a_in{h}") for h in range(2)]
        a2a_out = [dramp.tile([T, P], bf16, name=f"a2a_out{h}") for h in range(2)]

        for c in range(NCH):
            hp = ps.tile([P, 3 * P], fp32, tag="h", name="hp", bufs=3)
            ksp = hp[:, 0:P]
            wp = hp[:, P:2 * P]
            op_ = hp[:, 2 * P:3 * P]
            for i in range(NDT):
                nc.tensor.matmul(ksp, kq_all[:, i, c, 0:P], S_t[:, i],
                                 start=(i == 0), stop=(i == NDT - 1))
            rhsw = sb2.tile([P, P], bf16, tag="rhsw", name="rhsw")
            nc.vector.scalar_tensor_tensor(rhsw, ksp, rowfMn_col[:, c:c + 1],
                                           Vb[c], OP.mult, OP.add)
            nc.tensor.matmul(wp, TTs[c], rhsw, start=True, stop=True)
            W = sb2.tile([P, P], bf16, tag="W", name="W")
            nc.vector.tensor_copy(W, wp)
            W2 = sb2.tile([P, P], bf16, tag="W2", name="W2")
            nc.vector.tensor_scalar_mul(W2, wp, kbar_col[:, c:c + 1])
            # o = qf * (sum_d QT.T S + AmatT.T W)
            for i in range(NDT):
                nc.tensor.matmul(op_, kq_all[:, i, c, P:2 * P], S_t[:, i],
                                 start=(i == 0), stop=False)
            nc.tensor.matmul(op_, AmatT[c], W, start=False, stop=True)
            o_sb = sb2.tile([P, P], bf16, tag="osb", name="osb", bufs=3)
            nc.scalar.mul(o_sb, op_, qf_col[:, c:c + 1])
            h, j = c // 4, c % 4
            nc.sync.dma_start(a2a_in[h][j * P:(j + 1) * P, :], o_sb)
            nc.sync.dma_start(a2a_in[h][(j + 4) * P:(j + 5) * P, :], o_sb)
            # S update: S[i] = lam*S[i] + Knat[i].T @ W2
            sup = ps2.tile([P, 4 * P], fp32, tag="big", name="sup", bufs=3)
            for i in range(NDT):
                nc.tensor.matmul(sup[:, i * P:(i + 1) * P],
                                 Knat[c][:, i * P:(i + 1) * P], W2,
                                 start=True, stop=True)
            newS = sing.tile([P, NDT, P], bf16, tag=f"St_{c % 2}",
                             name=f"St_{c % 2}")
            nc.vector.scalar_tensor_tensor(
                newS, S_t, lamB[:, c:c + 1],
                sup.rearrange("p (i t) -> p i t", i=NDT), OP.mult, OP.add)
            S_t = newS
            if c == 3 or c == 7:
                nc.gpsimd.collective_compute(
                    kind="AllToAll", op=OP.bypass,
                    replica_groups=[[0, 1, 2, 3, 4, 5, 6, 7]],
                    ins=[a2a_in[c // 4][:]], outs=[a2a_out[c // 4][:]])

        # ── tail: RMSNorm + silu(gate) + output projection, per t-tile ──
        for tt in range(2):
            a2asb = sing.tile([P, 8, P], bf16, tag=f"a2asb{tt}",
                              name=f"a2asb{tt}")
            nc.sync.dma_start(
                a2asb, a2a_out[tt].rearrange("(i p) d -> p i d", i=8, p=P))
            # merge batch halves: oq = blk[b0]*m0 + blk[b1]*m1
            oq = sing.tile([P, 4, P], bf16, tag=f"oq{tt}", name=f"oq{tt}")
            tmpo = sb2.tile([P, 4, P], bf16, tag="tmpo", name="tmpo")
            nc.vector.tensor_scalar_mul(tmpo, a2asb[:, 4:8, :], bmB[:, 1:2])
            nc.vector.scalar_tensor_tensor(oq, a2asb[:, 0:4, :], bmB[:, 0:1],
                                           tmpo, OP.mult, OP.add)
            oqf = oq.rearrange("p i d -> p (i d)")
            junko = sb2.tile([P, 512], bf16, tag="junko", name="junko")
            ssq2 = sing.tile([P, 1], fp32, tag=f"ssq{tt}", name=f"ssq{tt}")
            nc.scalar.activation(junko, oqf, AF.Square,
                                 accum_out=ssq2[:, 0:1])
            srt = sing.tile([P, 1], fp32, tag=f"srt{tt}", name=f"srt{tt}")
            nc.scalar.activation(srt, ssq2, AF.Sqrt, bias=eps5c[:, 0:1],
                                 scale=1.0 / D)
            rsq = sing.tile([P, 1], fp32, tag=f"rsq{tt}", name=f"rsq{tt}")
            nc.vector.reciprocal(rsq, srt)
            # gate projection for this t-tile: out [t, d] layout
            gp = ps2.tile([P, 512], fp32, tag="big", name="gp", bufs=3)
            for kt in range(NDT):
                nc.tensor.matmul(gp, xq3[:, kt, tt * P:(tt + 1) * P],
                                 wg3[:, kt, :],
                                 start=(kt == 0), stop=(kt == NDT - 1))
            gsil = sb2.tile([P, 512], fp32, tag="gsil", name="gsil")
            if USE_SILU_TABLE:
                nc.scalar.activation(gsil, gp, AF.Silu)
            else:
                gsg = sb2.tile([P, 512], fp32, tag="gsg", name="gsg")
                nc.scalar.activation(gsg, gp, AF.Sigmoid)
                nc.vector.tensor_mul(gsil, gp, gsg)
            t1 = sb2.tile([P, 512], fp32, tag="t1", name="t1")
            nc.vector.scalar_tensor_tensor(t1, oqf, rsq[:, 0:1], wnormF,
                                           OP.mult, OP.mult)
            t3 = sb2.tile([P, 512], bf16, tag="t3", name="t3")
            nc.vector.tensor_mul(t3, t1, gsil)
            # transpose t3 -> [d, t] tiles
            t3T = sing.tile([P, NDT, P], bf16, tag=f"t3T{tt}", name=f"t3T{tt}")
            for h2 in range(2):
                tpt = ps2.tile([P, 2 * P], bf16, tag="pair", name="tpt")
                nc.tensor.transpose(tpt[:, 0:P], t3[:, (2 * h2) * P:(2 * h2 + 1) * P],
                                    ident_bf)
                nc.tensor.transpose(tpt[:, P:2 * P],
                                    t3[:, (2 * h2 + 1) * P:(2 * h2 + 2) * P],
                                    ident_bf)
                nc.vector.tensor_copy(
                    t3T.rearrange("p i d -> p (i d)")[:, h2 * 2 * P:(h2 + 1) * 2 * P],
                    tpt)
            # output projection: accumulate over d tiles
            po = ps2.tile([P, 512], fp32, tag="big", name="po", bufs=3)
            for i in range(NDT):
                nc.tensor.matmul(po, t3T[:, i], wo3[:, i, :],
                                 start=(i == 0), stop=(i == NDT - 1))
            ob = sb2.tile([P, 512], fp32, tag="ob", name="ob")
            nc.vector.tensor_copy(ob, po)
            nc.sync.dma_start(out_rows[tt * P:(tt + 1) * P, :], ob)


# ───────────────────────────── host-side prep ─────────────────────────────
def _tile512(a):
    # [512, N] -> [128, 4*N] with col = kt*N + j
    n = a.shape[1]
    return np.ascontiguousarray(
        a.reshape(NDT, P, n).transpose(1, 0, 2).reshape(P, NDT * n))


def prep_l1(x, q_proj_w, k_proj_w, v_proj_w, b_proj_w, a_proj_w, A_log,
            dt_bias, q_conv_w, k_conv_w, v_conv_w, g_proj_w=None,
            o_norm_w=None, o_proj_w=None):
    wqk = _tile512(np.concatenate([q_proj_w.T, k_proj_w.T], 1)).astype(BF16)
    sc = np.zeros((1, 8), np.float32)
    sc[0, 0] = float(dt_bias[0])
    sc[0, 1] = -float(np.exp(A_log[0]))
    xTs = [_tile512(np.ascontiguousarray(x[b].T)).astype(BF16) for b in range(B)]
    wgT = _tile512(np.ascontiguousarray(g_proj_w.T)).astype(BF16)
    woT = _tile512(np.ascontiguousarray(o_proj_w.T)).astype(BF16)
    wnr = np.ascontiguousarray(o_norm_w).reshape(1, 512).astype(np.float32)
    wba = _tile512(np.concatenate([b_proj_w.T, a_proj_w.T], 1)).astype(BF16)
    ins = []
    for b in range(B):
        for s in range(NDT):
            vsl = slice(s * P, (s + 1) * P)
            convw = np.zeros((P, 36), np.float32)
            for i in range(NDT):
                convw[:, 4 * i:4 * (i + 1)] = q_conv_w[i * P:(i + 1) * P]
                convw[:, 16 + 4 * i:16 + 4 * (i + 1)] = k_conv_w[i * P:(i + 1) * P]
            convw[:, 32:36] = v_conv_w[vsl]
            xb = x[b].T
            xqc = np.concatenate([xb[:, s * P:(s + 1) * P],
                                  xb[:, 512 + s * P:512 + (s + 1) * P]], 1)
            bm = np.zeros((1, 2), np.float32)
            bm[0, b] = 1.0
            ins.append({
                "xT": xTs[b],
                "wqk": wqk,
                "wv": _tile512(np.ascontiguousarray(v_proj_w.T[:, vsl])).astype(BF16),
                "wba": wba,
                "convw": convw,
                "sc": sc,
                "wg": wgT,
                "wo": woT,
                "wnr": wnr,
                "xq": _tile512(np.ascontiguousarray(xqc)).astype(BF16),
                "bmask": bm,
            })
    return ins


# ─────────────────────────── build + run (spmd) ───────────────────────────
def _build(kern, in_specs, out_specs):
    import concourse.mybir as mybir
    import concourse.tile as tile
    from concourse import bacc
    nc = bacc.Bacc(None, target_bir_lowering=False)
    with tile.TileContext(nc) as tc:
        with tc.tile_pool(name="io", bufs=1, space="DRAM") as io:
            ins = {k: io.tile(shape, dt, kind="ExternalInput", name=f"in_{k}")
                   for k, (shape, dt) in in_specs.items()}
            outs = {k: io.tile(shape, dt, kind="ExternalOutput", name=f"out_{k}")
                    for k, (shape, dt) in out_specs.items()}
            kern(tc, {k: v[:] for k, v in ins.items()},
                 {k: v[:] for k, v in outs.items()})
    nc.compile()
    return nc, ins, outs


_CACHE = {}


def _specs_l1():
    import concourse.mybir as mybir
    f, h = mybir.dt.float32, mybir.dt.bfloat16
    in_specs = {"xT": ((P, NDT * 1024), h), "wqk": ((P, NDT * 1024), h),
                "wv": ((P, NDT * P), h), "wba": ((P, NDT * 2), h),
                "convw": ((P, 36), f), "sc": ((1, 8), f),
                "wg": ((P, NDT * 512), h), "wo": ((P, NDT * 512), h),
                "wnr": ((1, 512), f), "xq": ((P, NDT * 256), h),
                "bmask": ((1, 2), f)}
    out_specs = {"out": ((2 * P, 512), f)}
    return in_specs, out_specs


def run_spmd(which, kern, specs, in_dicts, trace):
    from concourse.bass_utils import run_bass_kernel_spmd
    install_ntff_shim()
    if which not in _CACHE:
        _CACHE[which] = _build(kern, *specs)
    nc, ins, outs = _CACHE[which]
    in_maps = [{ins[k].name: np.ascontiguousarray(v) for k, v in d.items()}
               for d in in_dicts]
    t0 = time.perf_counter()
    try:
        res = run_bass_kernel_spmd(nc, in_maps, list(range(len(in_dicts))),
                                   trace=trace)
    except Exception:
        if not trace:
            raise
        res = run_bass_kernel_spmd(nc, in_maps, list(range(len(in_dicts))),
                                   trace=False)
    wall_ns = int((time.perf_counter() - t0) * 1e9)
    outl = [{k: np.asarray(res.results[c][outs[k].name])
             for k in outs} for c in range(len(in_dicts))]
    return outl, (res.exec_time_ns if res.exec_time_ns else wall_ns)


def kernel(x, q_proj_w, k_proj_w, v_proj_w, b_proj_w, a_proj_w, A_log,
           dt_bias, q_conv_w, k_conv_w, v_conv_w, g_proj_w, o_norm_w,
           o_proj_w, trace=True):
    args = [np.asarray(a, np.float32) for a in
            (x, q_proj_w, k_proj_w, v_proj_w, b_proj_w, a_proj_w, A_log,
             dt_bias, q_conv_w, k_conv_w, v_conv_w, g_proj_w, o_norm_w,
             o_proj_w)]
    (x, q_proj_w, k_proj_w, v_proj_w, b_proj_w, a_proj_w, A_log, dt_bias,
     q_conv_w, k_conv_w, v_conv_w, g_proj_w, o_norm_w, o_proj_w) = args

    ins1 = prep_l1(x, q_proj_w, k_proj_w, v_proj_w, b_proj_w, a_proj_w,
                   A_log, dt_bias, q_conv_w, k_conv_w, v_conv_w,
                   g_proj_w, o_norm_w, o_proj_w)
    r1, ns1 = run_spmd("l1", l1_kernel, _specs_l1(), ins1, trace)
    out = np.zeros((B, T, D), np.float32)
    for b in range(B):
        for s in range(NDT):
            r = r1[b * 4 + s]["out"].astype(np.float32)
            out[b, s * P:(s + 1) * P] = r[0:P]
            out[b, 512 + s * P:512 + (s + 1) * P] = r[P:2 * P]
    _LAST_HW_NS[0] = ns1
    return out
